# revision 15
# baseline (speedup 1.0000x reference)
"""IPAGNN Bass kernel for Trainium2, 8 NeuronCores.

Strategy (data-parallel over batch + vocab-sharded output projection):
  Phase 1 (one SPMD NEFF, 8 cores): core b runs example b's full 16-step
    graph propagation. States kept feature-major [H=128 partitions,
    nodes=256 free].  LSTM gate matmuls use the 128x128 weight blocks as
    lhsT (float32r: full-rate fp32 on the PE at free-dim>=256).  The
    scatter-add message passing is expressed as a dense matmul against an
    on-chip-built sparse weight matrix W^T[i,j] = p_t[i]*[t_i==j] +
    p_f[i]*[f_i==j] (built with iota/is_equal tensor_scalar ops).
    Per-example (steps[b], exit_index) handled branchlessly: all cores run
    16 steps; the exit-node column of the state is snapshotted into the
    output with a one-hot step gate.
  Host: gathers embedding rows (only 0.5MB of the 25MB table is needed),
    reassembles finals.
  Phase 2 (one SPMD NEFF, 8 cores): core v computes logits[:, shard_v] =
    finals @ out_W[:, shard_v] for all 8 examples (vocab-sharded).
"""

import sys

for _p in ("/opt/trn_rl_repo", "/opt/trn_rl_repo/concourse"):
    if _p not in sys.path:
        sys.path.insert(0, _p)

import numpy as np

import concourse.bass as bass
import concourse.tile as tile
from concourse import bacc, mybir
from concourse import bass_utils

FT = mybir.dt.float32
F32R = mybir.dt.float32r
AF = mybir.ActivationFunctionType
OP = mybir.AluOpType

B, N, L, H = 8, 256, 4, 128
LAYERS = 2
MAX_STEPS = 16
CONCAT = 2 * LAYERS * H  # 512
OUT_VOCAB = 30000
NCORES = 8
VSH = OUT_VOCAB // NCORES  # 3750 per-core vocab shard
NT = 2  # node tiles of 128

_cache = {}


def _r(ap):
    return ap.bitcast(F32R)


def _build_phase1(has_bias: bool, dbb: float):
    nc = bacc.Bacc("TRN2", target_bir_lowering=False, debug=False,
                   enable_asserts=False, num_devices=NCORES)

    def din(name, shape, dt=FT):
        return nc.dram_tensor(name, shape, dt, kind="ExternalInput").ap()

    embT_d = din("embT", [H, L * N], F32R)      # [:, t*256+j] = embed[data[b,j,t]]
    WX_d = din("WX", [H, 1024], F32R)           # concat(Wx[0], Wx[1]) along free
    WH_d = din("WH", [H, 1024], F32R)
    BRW_d = din("BRW", [H, 8], F32R)            # branch_W chunk k -> cols 2k:2k+2
    IDX_d = din("IDX", [H, 4])            # t0,t1,f0,f1 index columns (f32)
    IOTA_d = din("IOTA", [H, N])          # [p, j] = j
    ONESR_d = din("ONESR", [1, N])        # ones row
    ONESC_d = din("ONESC", [H, 1])        # ones column
    IDENT_d = din("IDENT", [H, H], F32R)        # identity for PE transpose
    IDENTF_d = din("IDENTF", [H, H])      # f32 identity (recip transpose)
    GATE_d = din("GATE", [H, MAX_STEPS])  # col s = 1.0 iff s == steps[b]-1
    EXM_d = din("EXM", [H, N])   # one-hot exit column mask
    BIASR_d = din("BIASR", [1, 1024])     # b as row (l-major, 128-per-gate)
    FIN_d = nc.dram_tensor("FIN", [H, 4 * N], FT, kind="ExternalOutput").ap()

    with tile.TileContext(nc) as tc:
        with (
            nc.allow_low_precision(reason="float32r matmul operands"),
            tc.tile_pool(name="const", bufs=1) as cp,
            tc.tile_pool(name="state", bufs=2) as sp,
            tc.tile_pool(name="tok", bufs=3) as tp,
            tc.tile_pool(name="elem", bufs=4) as ep,
            tc.tile_pool(name="wt", bufs=2) as wp,
            tc.tile_pool(name="psg", bufs=2, space="PSUM") as pg,
            tc.tile_pool(name="psmall", bufs=2, space="PSUM") as pm,
            tc.tile_pool(name="psagg", bufs=2, space="PSUM") as pa,
        ):
            # ---- load constants ----
            def load_const(dram, shape, dt=FT):
                t = cp.tile(shape, dt, tag=dram.name)
                nc.sync.dma_start(t[:], dram)
                return t

            embT = load_const(embT_d, [H, L * N], F32R)
            WX = load_const(WX_d, [H, 1024], F32R)
            WH = load_const(WH_d, [H, 1024], F32R)
            BRW = load_const(BRW_d, [H, 8], F32R)
            IDX = load_const(IDX_d, [H, 4])
            IOTA = load_const(IOTA_d, [H, N])
            ONESR = load_const(ONESR_d, [1, N])
            ONESC = load_const(ONESC_d, [H, 1])
            IDENT = load_const(IDENT_d, [H, H], F32R)
            IDENTF = load_const(IDENTF_d, [H, H])
            GATE = load_const(GATE_d, [H, MAX_STEPS])
            EXM = load_const(EXM_d, [H, N])
            BIASR = load_const(BIASR_d, [1, 1024]) if has_bias else None

            # ---- persistent state ----
            FINAL = cp.tile([H, 4 * N], FT, tag="final")
            nc.gpsimd.memset(FINAL[:], 0.0)
            IP = cp.tile([H, NT], FT, tag="ip")  # node-partition ip chunks
            nc.gpsimd.memset(IP[:], 0.0)
            nc.gpsimd.memset(IP[:1, 0:1], 1.0)

            cur_c = []
            cur_h = []
            for l in range(LAYERS):
                c0 = sp.tile([H, N], F32R, tag=f"c{l}")
                h0 = sp.tile([H, N], F32R, tag=f"h{l}")
                nc.gpsimd.memset(c0[:].bitcast(FT), 0.0)
                nc.gpsimd.memset(h0[:].bitcast(FT), 0.0)
                cur_c.append(c0)
                cur_h.append(h0)

            # ---- 16 steps ----
            for s in range(MAX_STEPS):
                # === LSTM over L tokens ===
                tc_c = list(cur_c)
                tc_h = list(cur_h)
                for t in range(L):
                    x = embT[:, t * N:(t + 1) * N]
                    for l in range(LAYERS):
                        G = pg.tile([H, 4 * N], FT, tag="g")
                        for q in range(4):  # i, f, o, g
                            gsl = G[:, q * N:(q + 1) * N]
                            nc.tensor.matmul(
                                gsl, WX[:, (l * 4 + q) * H:(l * 4 + q + 1) * H],
                                x, start=True, stop=False)
                            nc.tensor.matmul(
                                gsl, WH[:, (l * 4 + q) * H:(l * 4 + q + 1) * H],
                                tc_h[l][:], start=False, stop=not has_bias)
                            if has_bias:
                                nc.tensor.matmul(
                                    gsl,
                                    BIASR[:1, (l * 4 + q) * H:(l * 4 + q + 1) * H],
                                    ONESR[:1, :], start=False, stop=True)
                        SIG = ep.tile([H, 3 * N], FT, tag="sig")
                        nc.scalar.activation(SIG[:], G[:, :3 * N], AF.Sigmoid)
                        TG = ep.tile([H, N], FT, tag="tg")
                        nc.scalar.activation(TG[:], G[:, 3 * N:], AF.Tanh)
                        U1 = ep.tile([H, N], FT, tag="u1")
                        nc.vector.tensor_mul(U1[:], SIG[:, N:2 * N], tc_c[l][:])
                        U2 = ep.tile([H, N], FT, tag="u2")
                        nc.vector.tensor_mul(U2[:], SIG[:, 0:N], TG[:])
                        Cn = tp.tile([H, N], F32R, tag=f"ct{l}")
                        nc.vector.tensor_add(Cn[:], U1[:], U2[:])
                        TC = ep.tile([H, N], FT, tag="tc")
                        nc.scalar.activation(TC[:], Cn[:], AF.Tanh)
                        Hn = tp.tile([H, N], F32R, tag=f"ht{l}")
                        nc.vector.tensor_mul(Hn[:], SIG[:, 2 * N:3 * N], TC[:])
                        tc_c[l] = Cn
                        tc_h[l] = Hn
                        x = Hn[:]

                # === exit hold-back: blend step-input state at exit col ===
                for l in range(LAYERS):
                    for new, old in ((tc_c[l], cur_c[l]), (tc_h[l], cur_h[l])):
                        DD = ep.tile([H, N], FT, tag="exd")
                        nc.vector.tensor_sub(DD[:], old[:].bitcast(FT),
                                             new[:].bitcast(FT))
                        DM = ep.tile([H, N], FT, tag="exm2")
                        nc.vector.tensor_mul(DM[:], DD[:], EXM[:])
                        nc.vector.tensor_add(new[:], DM[:], new[:].bitcast(FT))
                S4 = [tc_c[0], tc_h[0], tc_c[1], tc_h[1]]  # concat order

                # === branch probs -> p_t, p_f  (node-partition layout) ===
                PT = ep.tile([H, NT], FT, tag="pt")
                PF = ep.tile([H, NT], FT, tag="pf")
                for c in range(NT):
                    BL = pm.tile([H, 2], FT, tag="ps")
                    for m in range(4):
                        nc.tensor.matmul(
                            BL[:, :2], S4[m][:, c * H:(c + 1) * H],
                            BRW[:, 2 * m:2 * m + 2],
                            start=(m == 0), stop=(m == 3))
                    BLs = ep.tile([H, 2], FT, tag="bls")
                    nc.scalar.copy(BLs[:], BL[:, :2])
                    D = ep.tile([H, 1], FT, tag="bd")
                    nc.vector.tensor_sub(D[:], BLs[:, 0:1], BLs[:, 1:2])
                    BD0 = ep.tile([H, 1], FT, tag="bd0")
                    nc.scalar.activation(BD0[:], D[:], AF.Sigmoid, bias=float(dbb))
                    nc.vector.tensor_mul(PT[:, c:c + 1], BD0[:], IP[:, c:c + 1])
                    nc.vector.tensor_sub(PF[:, c:c + 1], IP[:, c:c + 1],
                                         PT[:, c:c + 1])

                # === build W^T chunks  [i-part, j-free] ===
                WT = []
                for c in range(NT):
                    W1 = ep.tile([H, N], FT, tag="w1")
                    nc.vector.tensor_scalar(W1[:], IOTA[:], IDX[:, c:c + 1],
                                            PT[:, c:c + 1], OP.is_equal, OP.mult)
                    W2 = ep.tile([H, N], FT, tag="w2")
                    nc.vector.tensor_scalar(W2[:], IOTA[:], IDX[:, 2 + c:3 + c],
                                            PF[:, c:c + 1], OP.is_equal, OP.mult)
                    Wc = wp.tile([H, N], F32R, tag=f"wt{c}")
                    nc.vector.tensor_add(Wc[:], W1[:], W2[:])
                    WT.append(Wc)

                # === transpose states to node-major  ST[m][c] = S4[m][:,cH:]^T
                ST = [[None] * NT for _ in range(4)]
                for m in range(4):
                    for c in range(NT):
                        TPp = pm.tile([H, H], F32R, tag="ps")
                        nc.tensor.transpose(TPp[:], S4[m][:, c * H:(c + 1) * H],
                                            IDENT[:])
                        TPs = ep.tile([H, H], F32R, tag=f"st{m}{c}")
                        if (m + c) % 2 == 0:
                            nc.scalar.copy(TPs[:], TPp[:])
                        else:
                            nc.vector.tensor_copy(TPs[:], TPp[:])
                        ST[m][c] = TPs

                # === ip_new (both layouts) and 1/denom broadcast ===
                IPN = pm.tile([H, NT], FT, tag="ps")
                for c in range(NT):
                    for cc in range(NT):
                        nc.tensor.matmul(IPN[:, c:c + 1],
                                         WT[cc][:, c * H:(c + 1) * H].bitcast(FT),
                                         ONESC[:],
                                         start=(cc == 0), stop=(cc == 1))
                IPnew = cp.tile([H, NT], FT, tag="ipnew")
                nc.vector.tensor_copy(IPnew[:], IPN[:])
                RC = ep.tile([H, NT], FT, tag="rc")
                DEN = ep.tile([H, NT], FT, tag="den")
                nc.vector.tensor_scalar_add(DEN[:], IPN[:], 1e-7)
                nc.vector.reciprocal(RC[:], DEN[:])
                # transpose recip cols -> row [1, 256]
                RROW = ep.tile([1, N], FT, tag="rrow")
                for c in range(NT):
                    RT = pm.tile([1, H], FT, tag="ps")
                    nc.tensor.transpose(RT[:1, :], RC[:, c:c + 1], IDENTF[:])
                    nc.scalar.copy(RROW[:1, c * H:(c + 1) * H], RT[:1, :])
                RB = pm.tile([H, N], FT, tag="ps")
                nc.tensor.matmul(RB[:], ONESR[:1, :H], RROW[:1, :],
                                 start=True, stop=True)
                RBS = ep.tile([H, N], FT, tag="rbs")
                nc.scalar.copy(RBS[:], RB[:])

                # === aggregation matmuls + divide ===
                new_states = []
                for m in range(4):
                    AG = pa.tile([H, N], FT, tag="ag")
                    for c in range(NT):
                        nc.tensor.matmul(AG[:], ST[m][c][:], WT[c][:],
                                         start=(c == 0), stop=(c == 1))
                    tag = ("c0", "h0", "c1", "h1")[m]
                    Sn = sp.tile([H, N], F32R, tag=tag)
                    nc.vector.tensor_mul(Sn[:], AG[:], RBS[:])
                    new_states.append(Sn)

                # === snapshot full state gated by step (host picks column) ===
                for m in range(4):
                    nc.vector.scalar_tensor_tensor(
                        FINAL[:, m * N:(m + 1) * N], new_states[m][:].bitcast(FT),
                        GATE[:, s:s + 1], FINAL[:, m * N:(m + 1) * N],
                        OP.mult, OP.add)

                cur_c = [new_states[0], new_states[2]]
                cur_h = [new_states[1], new_states[3]]
                nc.vector.tensor_copy(IP[:], IPnew[:])

            nc.sync.dma_start(FIN_d, FINAL[:])

    nc.compile()
    return nc


def _build_phase2(has_obias: bool):
    nc = bacc.Bacc("TRN2", target_bir_lowering=False, debug=False,
                   enable_asserts=False, num_devices=NCORES)
    FINT_d = nc.dram_tensor("FINT", [H, 4 * B], F32R, kind="ExternalInput").ap()
    OWS_d = nc.dram_tensor("OWS", [H, 4 * VSH], F32R, kind="ExternalInput").ap()
    OB_d = nc.dram_tensor("OB", [1, VSH], F32R, kind="ExternalInput").ap()
    ONES8_d = nc.dram_tensor("ONES8", [1, B], F32R, kind="ExternalInput").ap()
    LOG_d = nc.dram_tensor("LOG", [B, VSH], FT, kind="ExternalOutput").ap()

    NTILE = 512
    with tile.TileContext(nc) as tc:
        with (
            nc.allow_low_precision(reason="float32r matmul operands"),
            tc.tile_pool(name="c2", bufs=1) as cp,
            tc.tile_pool(name="w2", bufs=6) as wpool,
            tc.tile_pool(name="o2", bufs=3) as opool,
            tc.tile_pool(name="p2", bufs=4, space="PSUM") as pp,
        ):
            FINT = cp.tile([H, 4 * B], F32R, tag="fint")
            nc.sync.dma_start(FINT[:], FINT_d)
            OB = cp.tile([1, VSH], F32R, tag="ob")
            ONES8 = cp.tile([1, B], F32R, tag="ones8")
            if has_obias:
                nc.sync.dma_start(OB[:1, :], OB_d)
                nc.sync.dma_start(ONES8[:1, :], ONES8_d)

            for off in range(0, VSH, NTILE):
                w = min(NTILE, VSH - off)
                WS = wpool.tile([H, 4 * NTILE], F32R, tag="ws")
                for k in range(4):
                    nc.sync.dma_start(WS[:, k * NTILE:k * NTILE + w],
                                      OWS_d[:, k * VSH + off:k * VSH + off + w])
                PS = pp.tile([B, NTILE], FT, tag="ps2")
                for k in range(4):
                    nc.tensor.matmul(PS[:, :w], FINT[:, k * B:(k + 1) * B],
                                     WS[:, k * NTILE:k * NTILE + w],
                                     start=(k == 0), stop=(k == 3 and not has_obias))
                if has_obias:
                    nc.tensor.matmul(PS[:, :w], ONES8[:1, :],
                                     OB[:1, off:off + w],
                                     start=False, stop=True)
                OUT = opool.tile([B, NTILE], FT, tag="out")
                nc.scalar.copy(OUT[:, :w], PS[:, :w])
                nc.sync.dma_start(LOG_d[:, off:off + w], OUT[:, :w])

    nc.compile()
    return nc


def _prep_phase1_inputs(data, true_idx, false_idx, exit_index, steps,
                        embed, Wx, Wh, b, branch_W):
    emb = embed[data]  # [B, N, L, H]
    in_maps = []
    iota = np.tile(np.arange(N, dtype=np.float32), (H, 1))
    ident = np.eye(H, dtype=np.float32)
    onesr = np.ones((1, N), np.float32)
    onesc = np.ones((H, 1), np.float32)
    # reference gate order is [i, f, g, o]; kernel wants [i, f, o, g]
    perm = np.r_[0:H, H:2 * H, 3 * H:4 * H, 2 * H:3 * H]
    WXh = np.concatenate([Wx[0][:, perm], Wx[1][:, perm]], axis=1).astype(np.float32)
    WHh = np.concatenate([Wh[0][:, perm], Wh[1][:, perm]], axis=1).astype(np.float32)
    BRWh = np.concatenate([branch_W[k * H:(k + 1) * H, :] for k in range(4)],
                          axis=1).astype(np.float32)
    BIASR = np.concatenate([b[0][perm], b[1][perm]])[None, :].astype(np.float32)
    for bb in range(B):
        embT = np.ascontiguousarray(
            emb[bb].transpose(2, 1, 0).reshape(H, L * N)).astype(np.float32)
        IDXm = np.stack([true_idx[bb, :H], true_idx[bb, H:],
                         false_idx[bb, :H], false_idx[bb, H:]],
                        axis=1).astype(np.float32)
        GATEm = np.zeros((H, MAX_STEPS), np.float32)
        GATEm[:, int(steps[bb]) - 1] = 1.0
        in_maps.append(dict(
            embT=embT, WX=WXh, WH=WHh, BRW=BRWh, IDX=IDXm, IOTA=iota,
            ONESR=onesr, ONESC=onesc, IDENT=ident, IDENTF=ident, GATE=GATEm,
            EXM=np.tile((np.arange(N) == int(exit_index[bb])
                         ).astype(np.float32), (H, 1)), BIASR=BIASR))
    return in_maps


def kernel(data, true_branch_nodes, false_branch_nodes, exit_index, steps,
           embed, Wx, Wh, b, branch_W, branch_b, out_W, out_b):
    data = np.asarray(data)
    steps = np.asarray(steps)
    exit_index = np.asarray(exit_index)
    has_bias = bool(np.any(b))
    dbb = float(branch_b[0] - branch_b[1])
    has_obias = bool(np.any(out_b))

    key1 = ("p1", has_bias, dbb)
    if key1 not in _cache:
        _cache[key1] = _build_phase1(has_bias, dbb)
    nc1 = _cache[key1]

    in_maps = _prep_phase1_inputs(data, np.asarray(true_branch_nodes),
                                  np.asarray(false_branch_nodes), exit_index,
                                  steps, np.asarray(embed), np.asarray(Wx),
                                  np.asarray(Wh), np.asarray(b),
                                  np.asarray(branch_W))
    res1 = bass_utils.run_bass_kernel_spmd(nc1, in_maps,
                                           core_ids=list(range(NCORES)))
    finals = np.stack([
        res1.results[bb]["FIN"].reshape(H, 4, N)[:, :, int(exit_index[bb])]
        .T.reshape(-1) for bb in range(B)
    ])  # [B, 512] in c0,h0,c1,h1 feature order

    # finals[b] layout must be [c0(128) h0(128) c1(128) h1(128)]
    # FIN is [H, 4] -> .T.reshape gives [4*H] with col-major order == correct.

    key2 = ("p2", has_obias)
    if key2 not in _cache:
        _cache[key2] = _build_phase2(has_obias)
    nc2 = _cache[key2]

    # FINT[:, k*B+b] = finals[b, k*128:(k+1)*128]
    FINT = np.ascontiguousarray(
        finals.reshape(B, 4, H).transpose(2, 1, 0)).reshape(H, 4 * B)
    ow4 = np.asarray(out_W).reshape(4, H, OUT_VOCAB)
    ones8 = np.ones((1, B), np.float32)
    in_maps2 = []
    for v in range(NCORES):
        sl = slice(v * VSH, (v + 1) * VSH)
        OWS = np.ascontiguousarray(
            ow4[:, :, sl].transpose(1, 0, 2).reshape(H, 4 * VSH)).astype(np.float32)
        OB = np.asarray(out_b)[sl][None, :].astype(np.float32)
        in_maps2.append(dict(FINT=FINT.astype(np.float32), OWS=OWS, OB=OB,
                             ONES8=ones8))
    res2 = bass_utils.run_bass_kernel_spmd(nc2, in_maps2,
                                           core_ids=list(range(NCORES)))
    logits = np.concatenate([res2.results[v]["LOG"] for v in range(NCORES)],
                            axis=1)
    return logits[:, None, :].astype(np.float32)
